# revision 8
# baseline (speedup 1.0000x reference)
"""Segment mean-pool (ContextEncoder) Bass kernel for Trainium2, 8 NeuronCores.

Problem: hidden_states [16, 4096, 1024] f32, output_mask [16, 4096] int
(non-decreasing segment ids per row in [0, 512), -1 = dropped token).
Output [16*512, 1024] f32: mean of tokens sharing (batch, segment id),
zeros for empty segments.

Strategy: data-parallel over batch, 2 rows per core. Per 128-token K-tile,
build a one-hot [tokens x segments] matrix on the vector engine (iota vs
per-partition segment id, is_equal), then accumulate one_hot.T @ x on the
tensor engine (fp16 operands, fp32 PSUM accumulate), one [128 seg x 1024]
PSUM region per 128-segment M-tile. Because ids are sorted, each K-tile
only touches 1-2 M-tiles; the (k -> M-tiles) map is computed on the host
from the actual masks (the program is compiled per input batch) so the
matmul count stays near the minimum while remaining exact for any mask
content. Mean = PSUM * (1/count) on an ACT-engine drain (counts from a
host bincount), written back as fp16 and upcast on the host.

Memory-system notes (measured on HW via in-NEFF loop + paired timing):
the per-core HBM ceiling is ~354-358 GB/s; a dependency-free replay of
all input reads + output writes (18.9 MB fp16) runs in 53.4us, which is
the traffic floor for this sharding. Reads alone: 49.6us (338 GB/s).
Writes in isolation run at ~360 GB/s (NOT 100-140 as previously noted),
and add their marginal byte cost when overlapped. The remaining gap to
the floor is diffuse: ~4us For_i iteration boundary (measured via 4-body
unroll: per-body 59.2us vs 63.1us per 1-body iteration), HBM write
receipt latency on the final tail write, and SBUF bank contention while
ACT drains PSUM against the DMA S2M stream. Mitigations here: mask and
1/count pack into ONE side tensor loaded per row on the out ring, so
the read ring starts data immediately; each segment m-tile is written
right after its drain; the last row's final m-tile drains and writes in
2 H-chunks (1KB DMA lines) so the serial tail after the last read is
small; the last row's final reads are single K-tiles; the host
pre-groups x into DMA order [NG, P, KPG, H] so every read DMA is one
fully contiguous 512KB HBM block (4KB/partition lines). PE chain
(~37us/core, fp16 matmuls at 1 col/cycle warm) and DVE one-hots stay
hidden under the DMA stream.
"""

import numpy as np

import concourse.bass as bass  # noqa: F401  (registers bass_rust)
import concourse.mybir as mybir
import concourse.tile as tile
from concourse import bacc
from concourse.bass_utils import run_bass_kernel_spmd

B, S, H = 16, 4096, 1024
NSEG = 512
NCORES = 8
RPC = B // NCORES          # rows (batch examples) per core
P = 128                    # partitions
KT = S // P                # 32 K-tiles of 128 tokens
MT = NSEG // P             # 4 M-tiles of 128 segments
NH = H // 512              # matmul free-dim chunks (PSUM bank = 512 f32)

F32 = mybir.dt.float32
F32R = mybir.dt.float32r   # full-rate fp32 matmul mode on TRN2
F16 = mybir.dt.float16
I8 = mybir.dt.int8

# "int8": global-scale symmetric quantization; halves input DMA traffic vs
#   fp16 (memory-bound win), on-chip upcast int8->fp16 split across
#   DVE/ACT/GPSIMD, ~6e-3 worst-case rel err (tolerance 2e-2).
# "fp16": half of fp32 input DMA traffic, ~2-4e-4 rel err.
# "fp32r": full fp32 traffic, ~1.6e-4 rel err.
PRECISION = "int8"

# int8 upcast split per full k-group (KPG k-tiles x 1024 H), as
# (engine, k_lo, k_hi, h_lo, h_hi) free-dim stripes. Loads per group:
# DVE 1024 elem (+one-hots), ACT 2048 (+drains), GPSIMD 1024.
UPC_STRIPES = [
    ("vector", 0, 2, 0, 512),
    ("scalar", 0, 2, 512, 1024),
    ("scalar", 2, 3, 0, 1024),
    ("gpsimd", 3, 4, 0, 1024),
]

# Number of SBUF buffers for data tiles (DMA prefetch depth)
DATA_BUFS = 10
D16_BUFS = 5       # upcast fp16 tile ring (int8 mode)
OH_BUFS = 32
OSB_BUFS = 2
KPG = 4            # k-tiles per input DMA: with the host-contiguous layout,
                   # 4 tiles = one 1MB contiguous HBM read (8KB/partition);
                   # interleaved A/B measured KPG=4 12.7us faster than KPG=2
IN_RING = "sync"   # HWDGE ring for input DMAs (dedicated: avoids head-of-line
OUT_RING = "scalar"  # blocking of inputs behind output DMAs waiting on drains)
MODE = "full"      # "full" | "dma_only" | "compute_only" | "no_out" | "out_only"
OUT_CHUNK = 1      # m-tiles per output DMA (out_only diagnostics)
OUT_ALT = False    # alternate output DMAs across both rings
OUT_FP16 = True    # device writes fp16 outputs (half the slow HBM write
                   # traffic); host upcasts to f32 after gather
OUT_COMBINE = True  # one output DMA per row instead of one per m-tile
OUT_SPLIT = 2       # with OUT_COMBINE: split the row write into this many DMAs
                    # (2 lets the first half issue after only 2 drains)
DRAIN_CHUNKS = 1    # H-chunks per non-final m-tile drain/write


def _build_program(klists, loop_n=1, precision=None, bodies_n=1):
    """klists[r][m] -> sorted list of K-tile indices whose token ids (in any
    row assigned to program slot r) overlap segment M-tile m. Must be
    non-empty for every (r, m).

    loop_n > 1 wraps the body in an in-NEFF repeat loop (timing only).
    bodies_n > 1 unrolls the body inside the loop (diagnostics)."""
    precision = precision or PRECISION
    if precision == "int8":
        ddt, mdt = I8, F16
    elif precision == "fp16":
        ddt = mdt = F16
    else:
        ddt = mdt = F32R
    nc = bacc.Bacc("TRN2", target_bir_lowering=False, debug=False)
    x = nc.dram_tensor("x", [RPC, KT // KPG, P, KPG, H], ddt,
                       kind="ExternalInput")
    side = nc.dram_tensor("side", [RPC, P, KT + MT], F32, kind="ExternalInput")
    odt = F16 if OUT_FP16 else F32
    out = nc.dram_tensor("out", [RPC, NSEG, H], odt, kind="ExternalOutput")

    with tile.TileContext(nc) as tc:
        with tc.tile_pool(name="const", bufs=1) as cpool, \
             tc.tile_pool(name="data", bufs=DATA_BUFS) as dpool, \
             tc.tile_pool(name="d16", bufs=D16_BUFS) as d16pool, \
             tc.tile_pool(name="oh", bufs=OH_BUFS) as opool, \
             tc.tile_pool(name="osb", bufs=OSB_BUFS) as spool, \
             tc.tile_pool(name="ps", bufs=MT, space="PSUM") as pspool:
            iota_t = cpool.tile([P, NSEG], F16, tag="iota")
            nc.gpsimd.iota(iota_t[:], [[1, NSEG]], channel_multiplier=0,
                           allow_small_or_imprecise_dtypes=True)
            body = _make_body(nc, klists, x, side, out, iota_t,
                              cpool, dpool, d16pool, opool, spool, pspool,
                              ddt, mdt, odt)
            if loop_n > 1:
                with tc.For_i(0, loop_n, 1):
                    for _ in range(bodies_n):
                        body()
            else:
                for _ in range(bodies_n):
                    body()
    nc.compile()
    return nc


def _make_body(nc, klists, x, side, out, iota_t,
               cpool, dpool, d16pool, opool, spool, pspool, ddt, mdt, odt):
    in_eng = getattr(nc, IN_RING)
    out_eng = getattr(nc, OUT_RING)
    int8 = ddt == I8
    upc_engs = {"vector": nc.vector, "scalar": nc.scalar, "gpsimd": nc.gpsimd}

    def upcast(d8, d16, g):
        """int8 -> fp16 on-chip, split across DVE/ACT/GPSIMD stripes."""
        if g == KPG:
            stripes = UPC_STRIPES
        else:  # tail singles: halve across the two fastest engines
            stripes = [("vector", 0, g, 0, 512), ("scalar", 0, g, 512, H)]
        for ename, klo, khi, hlo, hhi in stripes:
            eng = upc_engs[ename]
            if ename == "scalar":
                eng.activation(d16[:, klo:khi, hlo:hhi], d8[:, klo:khi, hlo:hhi],
                               mybir.ActivationFunctionType.Copy)
            else:
                eng.tensor_scalar(out=d16[:, klo:khi, hlo:hhi],
                                  in0=d8[:, klo:khi, hlo:hhi],
                                  scalar1=0.0, scalar2=None,
                                  op0=mybir.AluOpType.add)

    def body():
        # side inputs for BOTH rows up front, on the out ring: keeps the
        # input ring free so data reads start immediately at body entry
        sides = []
        for r in range(RPC):
            side_sb = cpool.tile([P, KT + MT], F32, tag=f"side{r}")
            out_eng.dma_start(out=side_sb[:], in_=side[r])
            sides.append(side_sb)
        for r in range(RPC):
            mask_sb = sides[r][:, 0:KT]
            invc_sb = sides[r][:, KT:KT + MT]

            k_to_ms = {}
            for m in range(MT):
                for k in klists[r][m]:
                    k_to_ms.setdefault(k, []).append(m)
            firsts = {m: klists[r][m][0] for m in range(MT)}
            lasts = {m: klists[r][m][-1] for m in range(MT)}

            psum = [pspool.tile([P, H], F32, tag="ps", name=f"psum_r{r}m{m}")
                    for m in range(MT)]

            if MODE in ("out_only", "dma_rw"):
                if MODE == "dma_rw":
                    for kg in range(KT // KPG):
                        dt0 = dpool.tile([P, KPG, H], ddt, tag="data",
                                         name=f"data_{r}_{kg}")
                        in_eng.dma_start(out=dt0[:], in_=x[r, kg])
                # OUT_CHUNK m-tiles per write DMA; alternate rings if OUT_ALT
                osb0 = spool.tile([P, MT, H], odt, tag="osb", name=f"osb_{r}")
                nc.vector.memset(osb0[:], 0.25)
                orv = out[r, :, :].rearrange("(m p) h -> p m h", p=P)
                for i, m in enumerate(range(0, MT, OUT_CHUNK)):
                    eng = (in_eng if (OUT_ALT and i % 2) else out_eng)
                    eng.dma_start(out=orv[:, m:m + OUT_CHUNK, :],
                                  in_=osb0[:, m:m + OUT_CHUNK, :])
                continue
            # host pre-groups x as [NG, P, KPG, H]: each group DMA is one
            # fully contiguous HBM block (KPG*H contiguous per partition)
            cdata = None
            kgroups = []
            for kg in range(KT // KPG):
                # last row: final group as single tiles so the tail matmul
                # chain starts as early as possible
                if r == RPC - 1 and kg == KT // KPG - 1:
                    kgroups += [(kg, c, 1) for c in range(KPG)]
                else:
                    kgroups.append((kg, 0, KPG))

            osb_row = spool.tile([P, MT, H], odt, tag="osb", name=f"osb_{r}")
            orv = out[r, :, :].rearrange("(m p) h -> p m h", p=P)

            def drain_m(m, fch):
                # drain on ACT so the DVE FIFO (one-hots) never queues
                # behind a drain that waits on matmuls; per-m writes issue
                # as soon as each m-tile drains. The very last m-tile of
                # the last row drains in H-chunks so the final HBM write
                # (the serial tail) is small.
                cw = H // fch
                for c in range(fch):
                    nc.scalar.activation(osb_row[:, m, c * cw:(c + 1) * cw],
                                         psum[m][:, c * cw:(c + 1) * cw],
                                         mybir.ActivationFunctionType.Copy,
                                         scale=invc_sb[:, m:m + 1])
                    if MODE != "no_out":
                        out_eng.dma_start(
                            out=orv[:, m:m + 1, c * cw:(c + 1) * cw],
                            in_=osb_row[:, m:m + 1, c * cw:(c + 1) * cw])

            # eager drains: drain each m-tile one k-group after its last
            # matmul was emitted (the one-group delay lets PE catch up so
            # ACT doesn't stall on the drain's matmul dependency); spreads
            # output writes across the whole row instead of a tail burst
            pending, done_ms = [], []
            for kg, c0, g in kgroups:
                k0 = kg * KPG + c0
                group = [k for k in range(k0, k0 + g) if k in k_to_ms]
                if not group:
                    continue
                if MODE == "compute_only":
                    if cdata is None:
                        cdata = dpool.tile([P, KPG, H], ddt, tag="data",
                                           name=f"data_{r}")
                        in_eng.dma_start(out=cdata[:], in_=x[r, 0])
                    data8 = cdata
                    g_eff = KPG
                else:
                    data8 = dpool.tile([P, g, H], ddt, tag="data",
                                       name=f"data_{r}_{k0}")
                    in_eng.dma_start(out=data8[:],
                                     in_=x[r, kg, :, c0:c0 + g, :])
                    g_eff = g
                if MODE == "dma_only":
                    continue
                if int8:
                    data_t = d16pool.tile([P, g_eff, H], mdt, tag="d16",
                                          name=f"d16_{r}_{k0}")
                    upcast(data8, data_t, g_eff)
                else:
                    data_t = data8
                for k in group:
                    ms = k_to_ms[k]
                    m0, span = ms[0], ms[-1] - ms[0] + 1
                    lite = MODE in ("no_mm", "no_oh")
                    mm_ms = [m for m in ms if not lite or k == firsts[m]]
                    if MODE == "no_oh" and not mm_ms:
                        continue
                    oh = opool.tile([P, span * P], mdt, tag="oh",
                                    name=f"oh_{r}_{k}")
                    nc.vector.tensor_scalar(
                        out=oh[:], in0=iota_t[:, m0 * P:(m0 + span) * P],
                        scalar1=mask_sb[:, k:k + 1],
                        scalar2=None, op0=mybir.AluOpType.is_equal)
                    for m in mm_ms:
                        for n in range(NH):
                            nc.tensor.matmul(
                                out=psum[m][:, n * 512:(n + 1) * 512],
                                lhsT=oh[:, (m - m0) * P:(m - m0 + 1) * P],
                                rhs=data_t[:, k - k0, n * 512:(n + 1) * 512],
                                start=(k == firsts[m]),
                                stop=(lite or k == lasts[m]))
                for m in pending:
                    drain_m(m, DRAIN_CHUNKS)
                    done_ms.append(m)
                pending = [m for m in range(MT)
                           if m not in done_ms and m not in pending
                           and lasts[m] <= group[-1]]
            if MODE == "dma_only":
                continue
            for m in pending:
                if r == RPC - 1 and m == MT - 1:
                    drain_m(m, 2)
                else:
                    drain_m(m, DRAIN_CHUNKS)
                done_ms.append(m)
    return body


def _prep(hidden_states, output_mask, precision=None):
    precision = precision or PRECISION
    hs = np.asarray(hidden_states)
    assert hs.shape == (B, S, H), hs.shape
    if precision == "int8":
        # symmetric global-scale quantization; the scale is folded into the
        # per-segment drain scale (invc) so the device math stays exact
        # integer sums in fp32 PSUM
        absmax = float(np.abs(hs).max())
        scale = absmax / 127.0 if absmax > 0 else 1.0
        q = np.rint(hs.astype(np.float32) * (1.0 / scale))
        np.clip(q, -127, 127, out=q)
        hs = q.astype(np.int8)
    else:
        scale = 1.0
        hs = hs.astype(np.float16 if precision == "fp16" else np.float32)
    # pre-group into the device DMA layout [B, NG, P, KPG, H]: each k-group
    # becomes one contiguous HBM block, read by a single descriptor-cheap DMA
    hs = np.ascontiguousarray(
        hs.reshape(B, KT // KPG, KPG, P, H).transpose(0, 1, 3, 2, 4))
    mask = np.asarray(output_mask).astype(np.int64)
    assert mask.shape == (B, S), mask.shape

    valid = mask >= 0
    # per-(row, K-tile) id range over valid tokens
    m3 = mask.reshape(B, KT, P)
    v3 = valid.reshape(B, KT, P)
    lo = np.where(v3, m3, np.iinfo(np.int64).max).min(axis=2)  # [B, KT]
    hi = np.where(v3, m3, -1).max(axis=2)                      # [B, KT]

    klists = []
    for r in range(RPC):
        rows = [c * RPC + r for c in range(NCORES)]
        per_m = []
        for m in range(MT):
            ks = [k for k in range(KT)
                  if any(lo[b, k] <= m * P + P - 1 and hi[b, k] >= m * P
                         for b in rows)]
            per_m.append(ks if ks else [0])
        klists.append(per_m)

    counts = np.zeros((B, NSEG), np.int64)
    for b in range(B):
        ids = mask[b][valid[b]]
        ids = ids[ids < NSEG]
        counts[b] = np.bincount(ids, minlength=NSEG)
    # drain scale: 1/count, with the int8 dequant scale folded in
    invc = (scale / np.maximum(counts, 1)).astype(np.float32)

    maskp = mask.astype(np.float32).reshape(B, KT, P).transpose(0, 2, 1)
    invcp = invc.reshape(B, MT, P).transpose(0, 2, 1)
    sidep = np.ascontiguousarray(np.concatenate([maskp, invcp], axis=2))

    in_maps = [{
        "x": hs[c * RPC:(c + 1) * RPC],
        "side": sidep[c * RPC:(c + 1) * RPC],
    } for c in range(NCORES)]
    return klists, in_maps


_PROGRAM_CACHE = {}


def _get_program(klists):
    key = (PRECISION,
           tuple(tuple(tuple(ks) for ks in per_m) for per_m in klists))
    if key not in _PROGRAM_CACHE:
        _PROGRAM_CACHE[key] = _build_program(klists)
    return _PROGRAM_CACHE[key]


def kernel(hidden_states, output_mask):
    klists, in_maps = _prep(hidden_states, output_mask)
    nc = _get_program(klists)
    res = run_bass_kernel_spmd(nc, in_maps, core_ids=list(range(NCORES)))
    full = np.concatenate(
        [res.results[c]["out"].reshape(RPC * NSEG, H).astype(np.float32)
         for c in range(NCORES)],
        axis=0)
    return full


if __name__ == "__main__":
    rng = np.random.default_rng(0)
    hs = rng.standard_normal((B, S, H)).astype(np.float32)
    mask = np.sort(rng.integers(0, NSEG, size=(B, S)), axis=-1).astype(np.int32)
    out = kernel(hidden_states=hs, output_mask=mask)
    print(out.shape, out.dtype)



# revision 9
# speedup vs baseline: 4.3774x; 4.3774x over previous
"""Segment mean-pool (ContextEncoder) Bass kernel for Trainium2, 8 NeuronCores.

Problem: hidden_states [16, 4096, 1024] f32, output_mask [16, 4096] int
(non-decreasing segment ids per row in [0, 512), -1 = dropped token).
Output [16*512, 1024] f32: mean of tokens sharing (batch, segment id),
zeros for empty segments.

Strategy: data-parallel over batch, 2 rows per core. Per 128-token K-tile,
build a one-hot [tokens x segments] matrix on the vector engine (iota vs
per-partition segment id, is_equal), then accumulate one_hot.T @ x on the
tensor engine (fp16 operands, fp32 PSUM accumulate), one [128 seg x 1024]
PSUM region per 128-segment M-tile. Because ids are sorted, each K-tile
only touches 1-2 M-tiles; the (k -> M-tiles) map is computed on the host
from the actual masks (the program is compiled per input batch) so the
matmul count stays near the minimum while remaining exact for any mask
content. Mean = PSUM * (1/count) on an ACT-engine drain (counts from a
host bincount), written back as fp16 and upcast on the host.

Memory-system notes (measured on HW via in-NEFF loop + paired timing):
the per-core HBM ceiling is ~354-358 GB/s; a dependency-free replay of
all input reads + output writes (18.9 MB fp16) runs in 53.4us, which is
the traffic floor for this sharding. Reads alone: 49.6us (338 GB/s).
Writes in isolation run at ~360 GB/s (NOT 100-140 as previously noted),
and add their marginal byte cost when overlapped. The remaining gap to
the floor is diffuse: ~4us For_i iteration boundary (measured via 4-body
unroll: per-body 59.2us vs 63.1us per 1-body iteration), HBM write
receipt latency on the final tail write, and SBUF bank contention while
ACT drains PSUM against the DMA S2M stream. Mitigations here: mask and
1/count pack into ONE side tensor loaded per row on the out ring, so
the read ring starts data immediately; each segment m-tile is written
right after its drain; the last row's final m-tile drains and writes in
2 H-chunks (1KB DMA lines) so the serial tail after the last read is
small; the last row's final reads are single K-tiles; the host
pre-groups x into DMA order [NG, P, KPG, H] so every read DMA is one
fully contiguous 512KB HBM block (4KB/partition lines). PE chain
(~37us/core, fp16 matmuls at 1 col/cycle warm) and DVE one-hots stay
hidden under the DMA stream.
"""

import numpy as np

import concourse.bass as bass  # noqa: F401  (registers bass_rust)
import concourse.mybir as mybir
import concourse.tile as tile
from concourse import bacc
from concourse.bass_utils import run_bass_kernel_spmd

B, S, H = 16, 4096, 1024
NSEG = 512
NCORES = 8
RPC = B // NCORES          # rows (batch examples) per core
P = 128                    # partitions
KT = S // P                # 32 K-tiles of 128 tokens
MT = NSEG // P             # 4 M-tiles of 128 segments
NH = H // 512              # matmul free-dim chunks (PSUM bank = 512 f32)

F32 = mybir.dt.float32
F32R = mybir.dt.float32r   # full-rate fp32 matmul mode on TRN2
F16 = mybir.dt.float16
I8 = mybir.dt.int8

# "int8": global-scale symmetric quantization; halves input DMA traffic vs
#   fp16 (memory-bound win), on-chip upcast int8->fp16 split across
#   DVE/ACT/GPSIMD, ~6e-3 worst-case rel err (tolerance 2e-2).
# "fp16": half of fp32 input DMA traffic, ~2-4e-4 rel err.
# "fp32r": full fp32 traffic, ~1.6e-4 rel err.
PRECISION = "int8"

# int8 upcast split per full k-group (KPG k-tiles x 1024 H), as
# (engine, k_lo, k_hi, h_lo, h_hi) free-dim stripes. Loads per group:
# DVE 1024 elem (+one-hots), ACT 2048 (+drains), GPSIMD 1024.
UPC_STRIPES = [
    ("vector", 0, 2, 0, 512),
    ("vector", 2, 4, 0, 512),
    ("scalar", 0, 2, 512, 1024),
    ("scalar", 2, 4, 512, 1024),
]

# Number of SBUF buffers for data tiles (DMA prefetch depth)
DATA_BUFS = 10
D16_BUFS = 5       # upcast fp16 tile ring (int8 mode)
OH_BUFS = 32
OSB_BUFS = 2
KPG = 4            # k-tiles per input DMA: with the host-contiguous layout,
                   # 4 tiles = one 1MB contiguous HBM read (8KB/partition);
                   # interleaved A/B measured KPG=4 12.7us faster than KPG=2
IN_RING = "sync"   # HWDGE ring for input DMAs (dedicated: avoids head-of-line
OUT_RING = "scalar"  # blocking of inputs behind output DMAs waiting on drains)
MODE = "full"      # "full" | "dma_only" | "compute_only" | "no_out" | "out_only"
OUT_CHUNK = 1      # m-tiles per output DMA (out_only diagnostics)
OUT_ALT = False    # alternate output DMAs across both rings
OUT_FP16 = True    # device writes fp16 outputs (half the slow HBM write
                   # traffic); host upcasts to f32 after gather
OUT_COMBINE = True  # one output DMA per row instead of one per m-tile
OUT_SPLIT = 2       # with OUT_COMBINE: split the row write into this many DMAs
                    # (2 lets the first half issue after only 2 drains)
DRAIN_CHUNKS = 1    # H-chunks per non-final m-tile drain/write


def _build_program(klists, loop_n=1, precision=None, bodies_n=1):
    """klists[r][m] -> sorted list of K-tile indices whose token ids (in any
    row assigned to program slot r) overlap segment M-tile m. Must be
    non-empty for every (r, m).

    loop_n > 1 wraps the body in an in-NEFF repeat loop (timing only).
    bodies_n > 1 unrolls the body inside the loop (diagnostics)."""
    precision = precision or PRECISION
    if precision == "int8":
        ddt, mdt = I8, F16
    elif precision == "fp16":
        ddt = mdt = F16
    else:
        ddt = mdt = F32R
    nc = bacc.Bacc("TRN2", target_bir_lowering=False, debug=False)
    x = nc.dram_tensor("x", [RPC, KT // KPG, P, KPG, H], ddt,
                       kind="ExternalInput")
    side = nc.dram_tensor("side", [RPC, P, KT + MT], F32, kind="ExternalInput")
    odt = F16 if OUT_FP16 else F32
    out = nc.dram_tensor("out", [RPC, NSEG, H], odt, kind="ExternalOutput")

    with tile.TileContext(nc) as tc:
        with tc.tile_pool(name="const", bufs=1) as cpool, \
             tc.tile_pool(name="data", bufs=DATA_BUFS) as dpool, \
             tc.tile_pool(name="d16", bufs=D16_BUFS) as d16pool, \
             tc.tile_pool(name="oh", bufs=OH_BUFS) as opool, \
             tc.tile_pool(name="osb", bufs=OSB_BUFS) as spool, \
             tc.tile_pool(name="ps", bufs=MT, space="PSUM") as pspool:
            iota_t = cpool.tile([P, NSEG], F16, tag="iota")
            nc.gpsimd.iota(iota_t[:], [[1, NSEG]], channel_multiplier=0,
                           allow_small_or_imprecise_dtypes=True)
            body = _make_body(nc, klists, x, side, out, iota_t,
                              cpool, dpool, d16pool, opool, spool, pspool,
                              ddt, mdt, odt)
            if loop_n > 1:
                with tc.For_i(0, loop_n, 1):
                    for _ in range(bodies_n):
                        body()
            else:
                for _ in range(bodies_n):
                    body()
    nc.compile()
    return nc


def _make_body(nc, klists, x, side, out, iota_t,
               cpool, dpool, d16pool, opool, spool, pspool, ddt, mdt, odt):
    in_eng = getattr(nc, IN_RING)
    out_eng = getattr(nc, OUT_RING)
    int8 = ddt == I8
    upc_engs = {"vector": nc.vector, "scalar": nc.scalar, "gpsimd": nc.gpsimd}

    def upcast(d8, d16, g):
        """int8 -> fp16 on-chip, split across DVE/ACT/GPSIMD stripes."""
        if g == KPG:
            stripes = UPC_STRIPES
        else:  # tail singles: halve across the two fastest engines
            stripes = [("vector", 0, g, 0, 512), ("scalar", 0, g, 512, H)]
        for ename, klo, khi, hlo, hhi in stripes:
            eng = upc_engs[ename]
            if ename == "scalar":
                eng.activation(d16[:, klo:khi, hlo:hhi], d8[:, klo:khi, hlo:hhi],
                               mybir.ActivationFunctionType.Copy)
            else:
                eng.tensor_scalar(out=d16[:, klo:khi, hlo:hhi],
                                  in0=d8[:, klo:khi, hlo:hhi],
                                  scalar1=0.0, scalar2=None,
                                  op0=mybir.AluOpType.add)

    def body():
        # side inputs for BOTH rows up front, on the out ring: keeps the
        # input ring free so data reads start immediately at body entry
        sides = []
        for r in range(RPC):
            side_sb = cpool.tile([P, KT + MT], F32, tag=f"side{r}")
            out_eng.dma_start(out=side_sb[:], in_=side[r])
            sides.append(side_sb)
        for r in range(RPC):
            mask_sb = sides[r][:, 0:KT]
            invc_sb = sides[r][:, KT:KT + MT]

            k_to_ms = {}
            for m in range(MT):
                for k in klists[r][m]:
                    k_to_ms.setdefault(k, []).append(m)
            firsts = {m: klists[r][m][0] for m in range(MT)}
            lasts = {m: klists[r][m][-1] for m in range(MT)}

            psum = [pspool.tile([P, H], F32, tag="ps", name=f"psum_r{r}m{m}")
                    for m in range(MT)]

            if MODE in ("out_only", "dma_rw"):
                if MODE == "dma_rw":
                    for kg in range(KT // KPG):
                        dt0 = dpool.tile([P, KPG, H], ddt, tag="data",
                                         name=f"data_{r}_{kg}")
                        in_eng.dma_start(out=dt0[:], in_=x[r, kg])
                # OUT_CHUNK m-tiles per write DMA; alternate rings if OUT_ALT
                osb0 = spool.tile([P, MT, H], odt, tag="osb", name=f"osb_{r}")
                nc.vector.memset(osb0[:], 0.25)
                orv = out[r, :, :].rearrange("(m p) h -> p m h", p=P)
                for i, m in enumerate(range(0, MT, OUT_CHUNK)):
                    eng = (in_eng if (OUT_ALT and i % 2) else out_eng)
                    eng.dma_start(out=orv[:, m:m + OUT_CHUNK, :],
                                  in_=osb0[:, m:m + OUT_CHUNK, :])
                continue
            # host pre-groups x as [NG, P, KPG, H]: each group DMA is one
            # fully contiguous HBM block (KPG*H contiguous per partition)
            cdata = None
            kgroups = []
            for kg in range(KT // KPG):
                # last row: final group as single tiles so the tail matmul
                # chain starts as early as possible
                if r == RPC - 1 and kg == KT // KPG - 1:
                    kgroups += [(kg, c, 1) for c in range(KPG)]
                else:
                    kgroups.append((kg, 0, KPG))

            osb_row = spool.tile([P, MT, H], odt, tag="osb", name=f"osb_{r}")
            orv = out[r, :, :].rearrange("(m p) h -> p m h", p=P)

            def drain_m(m, fch):
                # drain on ACT so the DVE FIFO (one-hots) never queues
                # behind a drain that waits on matmuls; per-m writes issue
                # as soon as each m-tile drains. The very last m-tile of
                # the last row drains in H-chunks so the final HBM write
                # (the serial tail) is small.
                cw = H // fch
                for c in range(fch):
                    nc.scalar.activation(osb_row[:, m, c * cw:(c + 1) * cw],
                                         psum[m][:, c * cw:(c + 1) * cw],
                                         mybir.ActivationFunctionType.Copy,
                                         scale=invc_sb[:, m:m + 1])
                    if MODE != "no_out":
                        out_eng.dma_start(
                            out=orv[:, m:m + 1, c * cw:(c + 1) * cw],
                            in_=osb_row[:, m:m + 1, c * cw:(c + 1) * cw])

            # eager drains: drain each m-tile one k-group after its last
            # matmul was emitted (the one-group delay lets PE catch up so
            # ACT doesn't stall on the drain's matmul dependency); spreads
            # output writes across the whole row instead of a tail burst
            pending, done_ms = [], []
            for kg, c0, g in kgroups:
                k0 = kg * KPG + c0
                group = [k for k in range(k0, k0 + g) if k in k_to_ms]
                if not group:
                    continue
                if MODE == "compute_only":
                    if cdata is None:
                        cdata = dpool.tile([P, KPG, H], ddt, tag="data",
                                           name=f"data_{r}")
                        in_eng.dma_start(out=cdata[:], in_=x[r, 0])
                    data8 = cdata
                    g_eff = KPG
                else:
                    data8 = dpool.tile([P, g, H], ddt, tag="data",
                                       name=f"data_{r}_{k0}")
                    in_eng.dma_start(out=data8[:],
                                     in_=x[r, kg, :, c0:c0 + g, :])
                    g_eff = g
                if MODE == "dma_only":
                    continue
                if int8:
                    data_t = d16pool.tile([P, g_eff, H], mdt, tag="d16",
                                          name=f"d16_{r}_{k0}")
                    upcast(data8, data_t, g_eff)
                else:
                    data_t = data8
                for k in group:
                    ms = k_to_ms[k]
                    m0, span = ms[0], ms[-1] - ms[0] + 1
                    lite = MODE in ("no_mm", "no_oh")
                    mm_ms = [m for m in ms if not lite or k == firsts[m]]
                    if MODE == "no_oh" and not mm_ms:
                        continue
                    oh = opool.tile([P, span * P], mdt, tag="oh",
                                    name=f"oh_{r}_{k}")
                    nc.vector.tensor_scalar(
                        out=oh[:], in0=iota_t[:, m0 * P:(m0 + span) * P],
                        scalar1=mask_sb[:, k:k + 1],
                        scalar2=None, op0=mybir.AluOpType.is_equal)
                    for m in mm_ms:
                        for n in range(NH):
                            nc.tensor.matmul(
                                out=psum[m][:, n * 512:(n + 1) * 512],
                                lhsT=oh[:, (m - m0) * P:(m - m0 + 1) * P],
                                rhs=data_t[:, k - k0, n * 512:(n + 1) * 512],
                                start=(k == firsts[m]),
                                stop=(lite or k == lasts[m]))
                for m in pending:
                    drain_m(m, DRAIN_CHUNKS)
                    done_ms.append(m)
                pending = [m for m in range(MT)
                           if m not in done_ms and m not in pending
                           and lasts[m] <= group[-1]]
            if MODE == "dma_only":
                continue
            for m in pending:
                if r == RPC - 1 and m == MT - 1:
                    drain_m(m, 2)
                else:
                    drain_m(m, DRAIN_CHUNKS)
                done_ms.append(m)
    return body


def _prep(hidden_states, output_mask, precision=None):
    precision = precision or PRECISION
    hs = np.asarray(hidden_states)
    assert hs.shape == (B, S, H), hs.shape
    if precision == "int8":
        # symmetric global-scale quantization; the scale is folded into the
        # per-segment drain scale (invc) so the device math stays exact
        # integer sums in fp32 PSUM
        absmax = float(np.abs(hs).max())
        scale = absmax / 127.0 if absmax > 0 else 1.0
        q = np.rint(hs.astype(np.float32) * (1.0 / scale))
        np.clip(q, -127, 127, out=q)
        hs = q.astype(np.int8)
    else:
        scale = 1.0
        hs = hs.astype(np.float16 if precision == "fp16" else np.float32)
    # pre-group into the device DMA layout [B, NG, P, KPG, H]: each k-group
    # becomes one contiguous HBM block, read by a single descriptor-cheap DMA
    hs = np.ascontiguousarray(
        hs.reshape(B, KT // KPG, KPG, P, H).transpose(0, 1, 3, 2, 4))
    mask = np.asarray(output_mask).astype(np.int64)
    assert mask.shape == (B, S), mask.shape

    valid = mask >= 0
    # per-(row, K-tile) id range over valid tokens
    m3 = mask.reshape(B, KT, P)
    v3 = valid.reshape(B, KT, P)
    lo = np.where(v3, m3, np.iinfo(np.int64).max).min(axis=2)  # [B, KT]
    hi = np.where(v3, m3, -1).max(axis=2)                      # [B, KT]

    klists = []
    for r in range(RPC):
        rows = [c * RPC + r for c in range(NCORES)]
        per_m = []
        for m in range(MT):
            ks = [k for k in range(KT)
                  if any(lo[b, k] <= m * P + P - 1 and hi[b, k] >= m * P
                         for b in rows)]
            per_m.append(ks if ks else [0])
        klists.append(per_m)

    counts = np.zeros((B, NSEG), np.int64)
    for b in range(B):
        ids = mask[b][valid[b]]
        ids = ids[ids < NSEG]
        counts[b] = np.bincount(ids, minlength=NSEG)
    # drain scale: 1/count, with the int8 dequant scale folded in
    invc = (scale / np.maximum(counts, 1)).astype(np.float32)

    maskp = mask.astype(np.float32).reshape(B, KT, P).transpose(0, 2, 1)
    invcp = invc.reshape(B, MT, P).transpose(0, 2, 1)
    sidep = np.ascontiguousarray(np.concatenate([maskp, invcp], axis=2))

    in_maps = [{
        "x": hs[c * RPC:(c + 1) * RPC],
        "side": sidep[c * RPC:(c + 1) * RPC],
    } for c in range(NCORES)]
    return klists, in_maps


_PROGRAM_CACHE = {}


def _get_program(klists):
    key = (PRECISION,
           tuple(tuple(tuple(ks) for ks in per_m) for per_m in klists))
    if key not in _PROGRAM_CACHE:
        _PROGRAM_CACHE[key] = _build_program(klists)
    return _PROGRAM_CACHE[key]


def kernel(hidden_states, output_mask):
    klists, in_maps = _prep(hidden_states, output_mask)
    nc = _get_program(klists)
    res = run_bass_kernel_spmd(nc, in_maps, core_ids=list(range(NCORES)))
    full = np.concatenate(
        [res.results[c]["out"].reshape(RPC * NSEG, H).astype(np.float32)
         for c in range(NCORES)],
        axis=0)
    return full


if __name__ == "__main__":
    rng = np.random.default_rng(0)
    hs = rng.standard_normal((B, S, H)).astype(np.float32)
    mask = np.sort(rng.integers(0, NSEG, size=(B, S)), axis=-1).astype(np.int32)
    out = kernel(hidden_states=hs, output_mask=mask)
    print(out.shape, out.dtype)



# revision 17
# speedup vs baseline: 6.2744x; 1.4334x over previous
"""Segment mean-pool (ContextEncoder) Bass kernel for Trainium2, 8 NeuronCores.

Problem: hidden_states [16, 4096, 1024] f32, output_mask [16, 4096] int
(non-decreasing segment ids per row in [0, 512), -1 = dropped token).
Output [16*512, 1024] f32: mean of tokens sharing (batch, segment id),
zeros for empty segments.

Strategy: data-parallel over batch, 2 rows per core. Per 128-token K-tile,
build a one-hot [tokens x segments] matrix on the vector engine (iota vs
per-partition segment id, is_equal), then accumulate one_hot.T @ x on the
tensor engine (fp16 operands, fp32 PSUM accumulate), one [128 seg x 1024]
PSUM region per 128-segment M-tile. Because ids are sorted, each K-tile
only touches 1-2 M-tiles; the (k -> M-tiles) map is computed on the host
from the actual masks (the program is compiled per input batch) so the
matmul count stays near the minimum while remaining exact for any mask
content. Mean = PSUM * (1/count) on an ACT-engine drain (counts from a
host bincount), written back as fp16 and upcast on the host.

Memory-system notes (measured on HW via in-NEFF loop + paired timing):
the per-core HBM ceiling is ~354-358 GB/s; a dependency-free replay of
all input reads + output writes (18.9 MB fp16) runs in 53.4us, which is
the traffic floor for this sharding. Reads alone: 49.6us (338 GB/s).
Writes in isolation run at ~360 GB/s (NOT 100-140 as previously noted),
and add their marginal byte cost when overlapped. The remaining gap to
the floor is diffuse: ~4us For_i iteration boundary (measured via 4-body
unroll: per-body 59.2us vs 63.1us per 1-body iteration), HBM write
receipt latency on the final tail write, and SBUF bank contention while
ACT drains PSUM against the DMA S2M stream. Mitigations here: mask and
1/count pack into ONE side tensor loaded per row on the out ring, so
the read ring starts data immediately; each segment m-tile is written
right after its drain; the last row's final m-tile drains and writes in
2 H-chunks (1KB DMA lines) so the serial tail after the last read is
small; the last row's final reads are single K-tiles; the host
pre-groups x into DMA order [NG, P, KPG, H] so every read DMA is one
fully contiguous 512KB HBM block (4KB/partition lines). PE chain
(~37us/core, fp16 matmuls at 1 col/cycle warm) and DVE one-hots stay
hidden under the DMA stream.
"""

import numpy as np

import concourse.bass as bass  # noqa: F401  (registers bass_rust)
import concourse.mybir as mybir
import concourse.tile as tile
from concourse import bacc
from concourse.bass_utils import run_bass_kernel_spmd

B, S, H = 16, 4096, 1024
NSEG = 512
NCORES = 8
RPC = B // NCORES          # rows (batch examples) per core
P = 128                    # partitions
KT = S // P                # 32 K-tiles of 128 tokens
MT = NSEG // P             # 4 M-tiles of 128 segments
NH = H // 512              # matmul free-dim chunks (PSUM bank = 512 f32)

F32 = mybir.dt.float32
F32R = mybir.dt.float32r   # full-rate fp32 matmul mode on TRN2
F16 = mybir.dt.float16
I8 = mybir.dt.int8

# "int8": global-scale symmetric quantization; halves input DMA traffic vs
#   fp16 (memory-bound win), on-chip upcast int8->fp16 split across
#   DVE/ACT/GPSIMD, ~6e-3 worst-case rel err (tolerance 2e-2).
# "fp16": half of fp32 input DMA traffic, ~2-4e-4 rel err.
# "fp32r": full fp32 traffic, ~1.6e-4 rel err.
PRECISION = "int8"

# int8 upcast split per full k-group (KPG k-tiles x 1024 H), as
# (engine, k_lo, k_hi, h_lo, h_hi) free-dim stripes. Loads per group:
# DVE 1024 elem (+one-hots), ACT 2048 (+drains), GPSIMD 1024.
UPC_STRIPES = [
    ("vector", 0, 2, 0, 1024),
    ("vector", 2, 3, 0, 1024),
    ("scalar", 3, 4, 0, 1024),
]

# Number of SBUF buffers for data tiles (DMA prefetch depth)
DATA_BUFS = 10
D16_BUFS = 5       # upcast fp16 tile ring (int8 mode)
OH_BUFS = 32
OSB_BUFS = 2
KPG = 4            # k-tiles per input DMA: with the host-contiguous layout,
                   # 4 tiles = one 1MB contiguous HBM read (8KB/partition);
                   # interleaved A/B measured KPG=4 12.7us faster than KPG=2
IN_RING = "sync"   # HWDGE ring for input DMAs (dedicated: avoids head-of-line
OUT_RING = "gpsimd"  # SWDGE ring for outputs: a DMA issued on an engine ring
                     # WAITS on that engine's in-order sequencer until its
                     # source data is ready, stalling every later instruction
                     # on that engine; Pool is otherwise idle so drain-waits
                     # cost nothing there
MODE = "full"      # "full" | "dma_only" | "compute_only" | "no_out" | "out_only"
OUT_CHUNK = 1      # m-tiles per output DMA (out_only diagnostics)
OUT_ALT = False    # alternate output DMAs across both rings
OUT_FP16 = True    # device writes fp16 outputs (half the slow HBM write
                   # traffic); host upcasts to f32 after gather
OUT_COMBINE = True  # one output DMA per row instead of one per m-tile
OUT_SPLIT = 2       # with OUT_COMBINE: split the row write into this many DMAs
                    # (2 lets the first half issue after only 2 drains)
DRAIN_CHUNKS = 1    # H-chunks per non-final m-tile drain/write
STAGGERED = False   # For_i(staggered_reset=True) deadlocks with SWDGE
                    # (Pool-ring) DMAs; instead amortize the loop barrier +
                    # lead-in/tail with a bodies_n unroll inside the loop


def _build_program(klists, loop_n=1, precision=None, bodies_n=1):
    """klists[r][m] -> sorted list of K-tile indices whose token ids (in any
    row assigned to program slot r) overlap segment M-tile m. Must be
    non-empty for every (r, m).

    loop_n > 1 wraps the body in an in-NEFF repeat loop (timing only).
    bodies_n > 1 unrolls the body inside the loop (diagnostics)."""
    precision = precision or PRECISION
    if precision == "int8":
        ddt, mdt = I8, F16
    elif precision == "fp16":
        ddt = mdt = F16
    else:
        ddt = mdt = F32R
    nc = bacc.Bacc("TRN2", target_bir_lowering=False, debug=False)
    x = nc.dram_tensor("x", [RPC, KT // KPG, P, KPG, H], ddt,
                       kind="ExternalInput")
    side = nc.dram_tensor("side", [RPC, P, KT + MT], F32, kind="ExternalInput")
    odt = F16 if OUT_FP16 else F32
    out = nc.dram_tensor("out", [RPC, NSEG, H], odt, kind="ExternalOutput")

    with tile.TileContext(nc) as tc:
        with tc.tile_pool(name="const", bufs=1) as cpool, \
             tc.tile_pool(name="data", bufs=DATA_BUFS) as dpool, \
             tc.tile_pool(name="d16", bufs=D16_BUFS) as d16pool, \
             tc.tile_pool(name="oh", bufs=OH_BUFS) as opool, \
             tc.tile_pool(name="osb", bufs=OSB_BUFS) as spool, \
             tc.tile_pool(name="ps", bufs=MT, space="PSUM") as pspool:
            iota_t = cpool.tile([P, NSEG], F16, tag="iota")
            nc.gpsimd.iota(iota_t[:], [[1, NSEG]], channel_multiplier=0,
                           allow_small_or_imprecise_dtypes=True)
            body = _make_body(nc, klists, x, side, out, iota_t,
                              cpool, dpool, d16pool, opool, spool, pspool,
                              ddt, mdt, odt)
            if loop_n > 1:
                with tc.For_i(0, loop_n, 1, staggered_reset=STAGGERED):
                    for _ in range(bodies_n):
                        body()
            else:
                for _ in range(bodies_n):
                    body()
    nc.compile()
    return nc


def _make_body(nc, klists, x, side, out, iota_t,
               cpool, dpool, d16pool, opool, spool, pspool, ddt, mdt, odt):
    in_eng = getattr(nc, IN_RING)
    out_eng = getattr(nc, OUT_RING)
    int8 = ddt == I8
    upc_engs = {"vector": nc.vector, "scalar": nc.scalar, "gpsimd": nc.gpsimd}

    def upcast(d8, d16, g):
        """int8 -> fp16 on-chip, split across DVE/ACT/GPSIMD stripes."""
        if g == KPG:
            stripes = UPC_STRIPES
        else:  # tail singles: halve across the two fastest engines
            stripes = [("vector", 0, g, 0, 512), ("scalar", 0, g, 512, H)]
        for ename, klo, khi, hlo, hhi in stripes:
            eng = upc_engs[ename]
            if ename == "scalar":
                eng.activation(d16[:, klo:khi, hlo:hhi], d8[:, klo:khi, hlo:hhi],
                               mybir.ActivationFunctionType.Copy)
            else:
                eng.tensor_scalar(out=d16[:, klo:khi, hlo:hhi],
                                  in0=d8[:, klo:khi, hlo:hhi],
                                  scalar1=0.0, scalar2=None,
                                  op0=mybir.AluOpType.add)

    def body():
        # side inputs for BOTH rows up front on the in ring (tiny: ~200ns
        # ahead of the first data group; the out ring's in-order sequencer
        # carries drain-waits that would delay them across bodies)
        sides = []
        for r in range(RPC):
            side_sb = cpool.tile([P, KT + MT], F32, tag=f"side{r}")
            in_eng.dma_start(out=side_sb[:], in_=side[r])
            sides.append(side_sb)
        for r in range(RPC):
            mask_sb = sides[r][:, 0:KT]
            invc_sb = sides[r][:, KT:KT + MT]

            k_to_ms = {}
            for m in range(MT):
                for k in klists[r][m]:
                    k_to_ms.setdefault(k, []).append(m)
            firsts = {m: klists[r][m][0] for m in range(MT)}
            lasts = {m: klists[r][m][-1] for m in range(MT)}

            psum = [pspool.tile([P, H], F32, tag="ps", name=f"psum_r{r}m{m}")
                    for m in range(MT)]

            if MODE in ("out_only", "dma_rw"):
                if MODE == "dma_rw":
                    for kg in range(KT // KPG):
                        dt0 = dpool.tile([P, KPG, H], ddt, tag="data",
                                         name=f"data_{r}_{kg}")
                        in_eng.dma_start(out=dt0[:], in_=x[r, kg])
                # OUT_CHUNK m-tiles per write DMA; alternate rings if OUT_ALT
                osb0 = spool.tile([P, MT, H], odt, tag="osb", name=f"osb_{r}")
                nc.vector.memset(osb0[:], 0.25)
                orv = out[r, :, :].rearrange("(m p) h -> p m h", p=P)
                for i, m in enumerate(range(0, MT, OUT_CHUNK)):
                    eng = (in_eng if (OUT_ALT and i % 2) else out_eng)
                    eng.dma_start(out=orv[:, m:m + OUT_CHUNK, :],
                                  in_=osb0[:, m:m + OUT_CHUNK, :])
                continue
            # host pre-groups x as [NG, P, KPG, H]: each group DMA is one
            # fully contiguous HBM block (KPG*H contiguous per partition)
            cdata = None
            kgroups = []
            for kg in range(KT // KPG):
                # last row: final group as single tiles so the tail matmul
                # chain starts as early as possible; first row: first group
                # as singles so the first upcast/matmul isn't gated on a
                # full 512KB DMA (shrinks the body lead-in)
                if (r == RPC - 1 and kg == KT // KPG - 1) or \
                   (r == 0 and kg == 0):
                    kgroups += [(kg, c, 1) for c in range(KPG)]
                else:
                    kgroups.append((kg, 0, KPG))

            osb_row = spool.tile([P, MT, H], odt, tag="osb", name=f"osb_{r}")
            orv = out[r, :, :].rearrange("(m p) h -> p m h", p=P)

            def drain_m(m, fch):
                # drain on ACT so the DVE FIFO (one-hots) never queues
                # behind a drain that waits on matmuls; per-m writes issue
                # as soon as each m-tile drains. The very last m-tile of
                # the last row drains in H-chunks so the final HBM write
                # (the serial tail) is small.
                cw = H // fch
                for c in range(fch):
                    nc.scalar.activation(osb_row[:, m, c * cw:(c + 1) * cw],
                                         psum[m][:, c * cw:(c + 1) * cw],
                                         mybir.ActivationFunctionType.Copy,
                                         scale=invc_sb[:, m:m + 1])
                    if MODE != "no_out":
                        out_eng.dma_start(
                            out=orv[:, m:m + 1, c * cw:(c + 1) * cw],
                            in_=osb_row[:, m:m + 1, c * cw:(c + 1) * cw])

            # eager drains: drain each m-tile one k-group after its last
            # matmul was emitted (the one-group delay lets PE catch up so
            # ACT doesn't stall on the drain's matmul dependency); spreads
            # output writes across the whole row instead of a tail burst
            pending, done_ms = [], []
            for kg, c0, g in kgroups:
                k0 = kg * KPG + c0
                group = [k for k in range(k0, k0 + g) if k in k_to_ms]
                if not group:
                    continue
                if MODE == "compute_only":
                    if cdata is None:
                        cdata = dpool.tile([P, KPG, H], ddt, tag="data",
                                           name=f"data_{r}")
                        in_eng.dma_start(out=cdata[:], in_=x[r, 0])
                    data8 = cdata
                    g_eff = KPG
                else:
                    data8 = dpool.tile([P, g, H], ddt, tag="data",
                                       name=f"data_{r}_{k0}")
                    in_eng.dma_start(out=data8[:],
                                     in_=x[r, kg, :, c0:c0 + g, :])
                    g_eff = g
                if MODE == "dma_only":
                    continue
                if int8:
                    data_t = d16pool.tile([P, g_eff, H], mdt, tag="d16",
                                          name=f"d16_{r}_{k0}")
                    upcast(data8, data_t, g_eff)
                else:
                    data_t = data8
                for k in group:
                    ms = k_to_ms[k]
                    m0, span = ms[0], ms[-1] - ms[0] + 1
                    lite = MODE in ("no_mm", "no_oh")
                    mm_ms = [m for m in ms if not lite or k == firsts[m]]
                    if MODE == "no_oh" and not mm_ms:
                        continue
                    oh = opool.tile([P, span * P], mdt, tag="oh",
                                    name=f"oh_{r}_{k}")
                    nc.vector.tensor_scalar(
                        out=oh[:], in0=iota_t[:, m0 * P:(m0 + span) * P],
                        scalar1=mask_sb[:, k:k + 1],
                        scalar2=None, op0=mybir.AluOpType.is_equal)
                    for m in mm_ms:
                        for n in range(NH):
                            nc.tensor.matmul(
                                out=psum[m][:, n * 512:(n + 1) * 512],
                                lhsT=oh[:, (m - m0) * P:(m - m0 + 1) * P],
                                rhs=data_t[:, k - k0, n * 512:(n + 1) * 512],
                                start=(k == firsts[m]),
                                stop=(lite or k == lasts[m]))
                for m in pending:
                    drain_m(m, DRAIN_CHUNKS)
                    done_ms.append(m)
                pending = [m for m in range(MT)
                           if m not in done_ms and m not in pending
                           and lasts[m] <= group[-1]]
            if MODE == "dma_only":
                continue
            for m in pending:
                if r == RPC - 1 and m == MT - 1:
                    drain_m(m, 2)
                else:
                    drain_m(m, DRAIN_CHUNKS)
                done_ms.append(m)
    return body


def _prep(hidden_states, output_mask, precision=None):
    precision = precision or PRECISION
    hs = np.asarray(hidden_states)
    assert hs.shape == (B, S, H), hs.shape
    if precision == "int8":
        # symmetric global-scale quantization; the scale is folded into the
        # per-segment drain scale (invc) so the device math stays exact
        # integer sums in fp32 PSUM
        absmax = float(np.abs(hs).max())
        scale = absmax / 127.0 if absmax > 0 else 1.0
        q = np.rint(hs.astype(np.float32) * (1.0 / scale))
        np.clip(q, -127, 127, out=q)
        hs = q.astype(np.int8)
    else:
        scale = 1.0
        hs = hs.astype(np.float16 if precision == "fp16" else np.float32)
    # pre-group into the device DMA layout [B, NG, P, KPG, H]: each k-group
    # becomes one contiguous HBM block, read by a single descriptor-cheap DMA
    hs = np.ascontiguousarray(
        hs.reshape(B, KT // KPG, KPG, P, H).transpose(0, 1, 3, 2, 4))
    mask = np.asarray(output_mask).astype(np.int64)
    assert mask.shape == (B, S), mask.shape

    valid = mask >= 0
    # per-(row, K-tile) id range over valid tokens
    m3 = mask.reshape(B, KT, P)
    v3 = valid.reshape(B, KT, P)
    lo = np.where(v3, m3, np.iinfo(np.int64).max).min(axis=2)  # [B, KT]
    hi = np.where(v3, m3, -1).max(axis=2)                      # [B, KT]

    klists = []
    for r in range(RPC):
        rows = [c * RPC + r for c in range(NCORES)]
        per_m = []
        for m in range(MT):
            ks = [k for k in range(KT)
                  if any(lo[b, k] <= m * P + P - 1 and hi[b, k] >= m * P
                         for b in rows)]
            per_m.append(ks if ks else [0])
        klists.append(per_m)

    counts = np.zeros((B, NSEG), np.int64)
    for b in range(B):
        ids = mask[b][valid[b]]
        ids = ids[ids < NSEG]
        counts[b] = np.bincount(ids, minlength=NSEG)
    # drain scale: 1/count, with the int8 dequant scale folded in
    invc = (scale / np.maximum(counts, 1)).astype(np.float32)

    maskp = mask.astype(np.float32).reshape(B, KT, P).transpose(0, 2, 1)
    invcp = invc.reshape(B, MT, P).transpose(0, 2, 1)
    sidep = np.ascontiguousarray(np.concatenate([maskp, invcp], axis=2))

    in_maps = [{
        "x": hs[c * RPC:(c + 1) * RPC],
        "side": sidep[c * RPC:(c + 1) * RPC],
    } for c in range(NCORES)]
    return klists, in_maps


_PROGRAM_CACHE = {}


def _get_program(klists):
    key = (PRECISION,
           tuple(tuple(tuple(ks) for ks in per_m) for per_m in klists))
    if key not in _PROGRAM_CACHE:
        _PROGRAM_CACHE[key] = _build_program(klists)
    return _PROGRAM_CACHE[key]


def kernel(hidden_states, output_mask):
    klists, in_maps = _prep(hidden_states, output_mask)
    nc = _get_program(klists)
    res = run_bass_kernel_spmd(nc, in_maps, core_ids=list(range(NCORES)))
    full = np.concatenate(
        [res.results[c]["out"].reshape(RPC * NSEG, H).astype(np.float32)
         for c in range(NCORES)],
        axis=0)
    return full


if __name__ == "__main__":
    rng = np.random.default_rng(0)
    hs = rng.standard_normal((B, S, H)).astype(np.float32)
    mask = np.sort(rng.integers(0, NSEG, size=(B, S)), axis=-1).astype(np.int32)
    out = kernel(hidden_states=hs, output_mask=mask)
    print(out.shape, out.dtype)

